# revision 1
# baseline (speedup 1.0000x reference)
"""Trainium2 Bass kernel for nn_ExperimentalLayer9 (dense transformer layer).

Layer: x + gelu(attn(x) ) @ Wf with
  Q = split_heads(x), K = split_heads(x@Wk+bk), V = split_heads(x@Wv+bv)
  causal softmax (no 1/sqrt(d) scale), exact-erf gelu, residual add.

Sharding over 8 NeuronCores: 2 batch groups x 4-way head/tensor parallel.
Core c handles batch b=c//4 and heads [4r, 4r+4) with r=c%4.  Each core
computes K^T/V projections for its head slice, causal flash-style
attention in transposed-score layout, gelu, and a partial FF over its
1024-row slice of Wf.  A 4-rank ReduceScatter (bf16) sums the FF
partials within each batch group; each core adds the residual x rows for
its rank's 512-row shard and returns that shard.  The host reassembles
the [2, 2048, 1024] output.

All matmuls run in bf16 (fp32 PSUM accumulation); softmax/normalization
in fp32.  exp is computed without max-subtraction (scores are bounded:
std ~5, so exp stays well inside fp32/bf16 range) which avoids any
partition-axis max reduction.  The exp-sum l(q) is obtained for free by
appending a ones-column to V in the attention@V matmul; 1/l is then a
per-partition scalar multiply fused on the vector engine.
"""

import numpy as np
import ml_dtypes

import concourse.bass as bass
import concourse.mybir as mybir
import concourse.tile as tile
from concourse import bacc
from concourse import bass_utils

# Problem shapes (hardcoded per contest contract).
B, S, D, H, DHID = 2, 2048, 1024, 16, 4096
NCORES = 8
GROUP = 4              # cores per batch group
HPC = 4                # heads per core
DK = 64                # q/k head dim
DV = 256               # v head dim
DKS = HPC * DK         # 256  k-slice per core
DVS = HPC * DV         # 1024 v/hidden slice per core
ROWS = S // GROUP      # 512  output rows per core after ReduceScatter
NM = D // 128          # 8    contraction chunks over d_model
VSTRIDE = DV + 1       # 257  V columns per head incl. ones column

BF16 = mybir.dt.bfloat16
F32 = mybir.dt.float32
AF = mybir.ActivationFunctionType

bf16 = ml_dtypes.bfloat16

_compiled = None


def build_program():
    nc = bacc.Bacc(
        "TRN2",
        target_bir_lowering=False,
        debug=False,
        enable_asserts=True,
        num_devices=NCORES,
    )

    # Per-core inputs (values differ per core; program is SPMD-identical).
    xT = nc.dram_tensor("xT", [D, S], BF16, kind="ExternalInput").ap()
    qT = nc.dram_tensor("qT", [DKS, S], BF16, kind="ExternalInput").ap()
    xres = nc.dram_tensor("xres", [ROWS, D], F32, kind="ExternalInput").ap()
    wk = nc.dram_tensor("wk", [D, DKS], BF16, kind="ExternalInput").ap()
    wv = nc.dram_tensor("wv", [D, DVS], BF16, kind="ExternalInput").ap()
    wf = nc.dram_tensor("wf", [DVS, D], BF16, kind="ExternalInput").ap()
    bkb = nc.dram_tensor("bkb", [1, DKS], BF16, kind="ExternalInput").ap()
    bvb = nc.dram_tensor("bvb", [1, DVS], BF16, kind="ExternalInput").ap()
    maskt = nc.dram_tensor("maskt", [128, 128], BF16, kind="ExternalInput").ap()
    ident = nc.dram_tensor("ident", [128, 128], BF16, kind="ExternalInput").ap()
    onesr = nc.dram_tensor("onesr", [1, 512], BF16, kind="ExternalInput").ap()
    out = nc.dram_tensor("out", [ROWS, D], F32, kind="ExternalOutput").ap()

    with tile.TileContext(nc) as tc:
        _body(nc, tc, xT, qT, xres, wk, wv, wf, bkb, bvb, maskt, ident, onesr, out)

    nc.compile()
    return nc


def _body(nc, tc, xT, qT, xres, wk, wv, wf, bkb, bvb, maskt, ident, onesr, out):
    NST = S // 128     # 16 s tiles of 128
    NQT2 = S // 1024   # 2  q tiles of 1024

    with (
        tc.tile_pool(name="const", bufs=1) as constp,
        tc.tile_pool(name="kv", bufs=1) as kvp,
        tc.tile_pool(name="got", bufs=1) as gotp,
        tc.tile_pool(name="res", bufs=1) as resp,
        tc.tile_pool(name="rfp", bufs=2) as rfp,
        tc.tile_pool(name="small", bufs=8) as smallp,
        tc.tile_pool(name="dram", bufs=1, space="DRAM") as dramp,
    ):
        # ---- constants (ACT queue) ------------------------------------
        ones_sb = constp.tile([1, 512], BF16)
        nc.scalar.dma_start(ones_sb[:], onesr[:])
        mask_sb = constp.tile([128, 128], BF16)
        nc.scalar.dma_start(mask_sb[:], maskt[:])
        bk_sb = constp.tile([1, DKS], BF16)
        nc.scalar.dma_start(bk_sb[:], bkb[:])
        bv_sb = constp.tile([1, DVS], BF16)
        nc.scalar.dma_start(bv_sb[:], bvb[:])

        # Warm up the collectives path (ncfw/channel setup) so the first
        # real ReduceScatter doesn't pay ~25us of first-call overhead.
        warm_in = dramp.tile([4, 16], BF16, tag="warm_in")
        warm_out = dramp.tile([1, 16], BF16, tag="warm_out")
        nc.scalar.dma_start(
            warm_in[:].rearrange("a b -> (a b)")[None, :], ones_sb[0:1, 0:64]
        )
        nc.gpsimd.collective_compute(
            "ReduceScatter",
            mybir.AluOpType.add,
            replica_groups=[[0, 1, 2, 3], [4, 5, 6, 7]],
            ins=[warm_in.opt()],
            outs=[warm_out.opt()],
        )

        # [1024, n] DRAM -> [128, 8*n] SBUF, per-chunk DMAs on the Sync
        # queue (all complete before the first xbar transpose issues)
        def load_chunked(pool, src, n):
            t = pool.tile([128, NM * n], src.dtype)
            for m in range(NM):
                nc.sync.dma_start(
                    t[:, m * n : (m + 1) * n],
                    src[m * 128 : (m + 1) * 128, :],
                )
            return t

        qT_sb = kvp.tile([128, 2 * S], BF16)
        for m in range(2):
            nc.sync.dma_start(
                qT_sb[:, m * S : (m + 1) * S], qT[m * 128 : (m + 1) * 128, :]
            )
        kt_sb = kvp.tile([128, 2 * S], BF16)   # K^T rows dk%128, chunk dk//128
        v_sb = kvp.tile([128, NST * HPC * VSTRIDE], BF16)
        got_sb = gotp.tile([128, NM * S], BF16)  # gelu(o)^T, hc-major x q
        # residual x rows: no deps, load early (ACT queue)
        xrs = []
        for g in range(4):
            xr = resp.tile([128, D], F32, tag=f"xr{g}")
            nc.scalar.dma_start(xr[:], xres[g * 128 : (g + 1) * 128, :])
            xrs.append(xr)

        # ---- projections ---------------------------------------------
        with (
            tc.tile_pool(name="projw", bufs=1) as pwp,
            tc.tile_pool(name="xt", bufs=1) as xtp,
            tc.tile_pool(name="psProj", bufs=4, space="PSUM") as psP,
        ):
            wk_sb = load_chunked(pwp, wk, DKS)
            xT_sb = load_chunked(xtp, xT, S)
            wv_sb = load_chunked(pwp, wv, DVS)

            # K^T[dk, s]: lhsT = Wk chunk [128m, 128dk], rhs = xT chunk [128m, 512s]
            for dkt in range(2):
                for st in range(4):
                    ps = psP.tile([128, 512], F32, tag="proj")
                    nc.tensor.matmul(
                        ps[:],
                        bk_sb[:, dkt * 128 : (dkt + 1) * 128],
                        ones_sb[:, 0:512],
                        start=True,
                        stop=False,
                    )
                    for m in range(NM):
                        nc.tensor.matmul(
                            ps[:],
                            wk_sb[:, m * DKS + dkt * 128 : m * DKS + dkt * 128 + 128],
                            xT_sb[:, m * S + st * 512 : m * S + st * 512 + 512],
                            start=False,
                            stop=(m == NM - 1),
                        )
                    nc.scalar.copy(
                        kt_sb[:, dkt * S + st * 512 : dkt * S + st * 512 + 512], ps[:]
                    )

            # V[s, dv] with a ones column per head (col 256 of each strip)
            nc.vector.memset(
                v_sb[:].rearrange("p (t h c) -> p t h c", t=NST, h=HPC)[:, :, :, DV],
                1.0,
            )
            for st in range(NST):
                for dvh in range(2):  # dv halves of 512 = heads (2*dvh, 2*dvh+1)
                    ps = psP.tile([128, 512], F32, tag="proj")
                    nc.tensor.matmul(
                        ps[:],
                        ones_sb[:, 0:128],
                        bv_sb[:, dvh * 512 : dvh * 512 + 512],
                        start=True,
                        stop=False,
                    )
                    for m in range(NM):
                        nc.tensor.matmul(
                            ps[:],
                            xT_sb[:, m * S + st * 128 : m * S + st * 128 + 128],
                            wv_sb[:, m * DVS + dvh * 512 : m * DVS + dvh * 512 + 512],
                            start=False,
                            stop=(m == NM - 1),
                        )
                    base = st * HPC * VSTRIDE
                    for hh in range(2):
                        h = 2 * dvh + hh
                        nc.scalar.copy(
                            v_sb[:, base + h * VSTRIDE : base + h * VSTRIDE + DV],
                            ps[:, hh * 256 : hh * 256 + 256],
                        )

        # ---- attention (head pairs, row-tiled scores) ----------------
        # scores^T[k, q]: contraction is dk=64, so heads 2p (PE rows 0-63)
        # and 2p+1 (rows 64-127) run concurrently via tile_position row
        # tiling.  AV groups run in default 128x128 mode afterwards;
        # exp without max-subtraction; o tiles transposed by xbar DMA.
        with (
            tc.tile_pool(name="expp", bufs=1) as expp,
            tc.tile_pool(name="otile", bufs=4) as otp,
            tc.tile_pool(name="psSt", bufs=3, space="PSUM") as psS,
            tc.tile_pool(name="psAv", bufs=2, space="PSUM") as psV,
        ):
            NQT2 = S // 1024
            for pair in range(2):
                co = pair * S           # both heads of the pair share chunk co

                def st_tile(j, kt, hl, exps):
                    po = 64 * (hl % 2)
                    t = kt - 8 * j   # >=0 on diagonal k-tiles
                    toff = max(t, 0) * 128
                    q0 = j * 1024 + toff
                    ps = psS.tile([128, 1024], F32, tag="st")
                    lo_w = max(0, 512 - toff)
                    if lo_w:
                        nc.tensor.matmul(
                            ps[:, toff : toff + lo_w],
                            kt_sb[po : po + 64, co + kt * 128 : co + kt * 128 + 128],
                            qT_sb[po : po + 64, co + q0 : co + q0 + lo_w],
                            start=True,
                            stop=True,
                            tile_position=(po, 0),
                        )
                    nc.tensor.matmul(
                        ps[:, max(toff, 512) : 1024],
                        kt_sb[po : po + 64, co + kt * 128 : co + kt * 128 + 128],
                        qT_sb[po : po + 64, co + j * 1024 + max(toff, 512) : co + (j + 1) * 1024],
                        start=True,
                        stop=True,
                        tile_position=(po, 0),
                    )
                    nc.scalar.activation(
                        exps[:, kt * 1024 + toff : (kt + 1) * 1024],
                        ps[:, toff:1024],
                        AF.Exp,
                    )
                    if t >= 0:  # mask the diagonal 128x128 block
                        blk = exps[:, kt * 1024 + toff : kt * 1024 + toff + 128]
                        nc.vector.tensor_mul(blk, blk, mask_sb[:])

                def av_tile(j, sq, hl, exps):
                    i = 8 * j + sq
                    pso = psV.tile([128, VSTRIDE], F32, tag="av")
                    for kt in range(i + 1):
                        vb = kt * HPC * VSTRIDE + hl * VSTRIDE
                        nc.tensor.matmul(
                            pso[:],
                            exps[:, kt * 1024 + sq * 128 : kt * 1024 + sq * 128 + 128],
                            v_sb[:, vb : vb + VSTRIDE],
                            start=(kt == 0),
                            stop=(kt == i),
                        )
                    recip = smallp.tile([128, 1], F32, tag="recip")
                    nc.vector.reciprocal(recip[:], pso[:, DV : DV + 1])
                    ot = otp.tile([128, DV], BF16, tag="ot")
                    nc.vector.tensor_scalar_mul(ot[:], pso[:, 0:DV], recip[:])
                    for half in range(2):
                        hc = 2 * hl + half
                        nc.sync.dma_start_transpose(
                            got_sb[:, hc * S + i * 128 : hc * S + i * 128 + 128],
                            ot[:, half * 128 : half * 128 + 128],
                        )

                for j in range(NQT2):   # 1024-wide q tiles
                    hA, hB = 2 * pair, 2 * pair + 1
                    exps_a = expp.tile([128, 16 * 1024], BF16, tag="expSA")
                    exps_b = expp.tile([128, 16 * 1024], BF16, tag="expSB")
                    # row-tiled score phase: both heads stream concurrently
                    for kt in range(8 * j + 8):
                        st_tile(j, kt, hA, exps_a)
                        st_tile(j, kt, hB, exps_b)
                    # default-mode AV phase
                    for sq in range(8):
                        av_tile(j, sq, hA, exps_a)
                        av_tile(j, sq, hB, exps_b)

        # ---- gelu (exact erf) in place on transposed layout ----------
        for hc in range(NM):
            nc.scalar.activation(
                got_sb[:, hc * S : (hc + 1) * S],
                got_sb[:, hc * S : (hc + 1) * S],
                AF.Gelu,
            )

        # ---- FF partial + chunked ReduceScatter + gpsimd residual ----
        with (
            tc.tile_pool(name="ffw", bufs=1) as ffwp,
            tc.tile_pool(name="ffout", bufs=4) as ffoutp,
            tc.tile_pool(name="psFf", bufs=3, space="PSUM") as psF,
        ):
            wf_sb = load_chunked(ffwp, wf, D)
            for g in range(4):
                partial_d = dramp.tile([512, D], BF16, tag=f"part{g}")
                for cc in range(4):
                    c = 4 * g + cc
                    ps0 = psF.tile([128, 512], F32, tag="ff0")
                    ps1 = psF.tile([128, 512], F32, tag="ff1")
                    for hc in range(NM):
                        lhsT = got_sb[:, hc * S + c * 128 : hc * S + c * 128 + 128]
                        nc.tensor.matmul(
                            ps0[:], lhsT, wf_sb[:, hc * D : hc * D + 512],
                            start=(hc == 0), stop=(hc == NM - 1),
                        )
                        nc.tensor.matmul(
                            ps1[:], lhsT, wf_sb[:, hc * D + 512 : hc * D + 1024],
                            start=(hc == 0), stop=(hc == NM - 1),
                        )
                    fo = ffoutp.tile([128, D], BF16, tag="ffout")
                    nc.vector.tensor_copy(fo[:, 0:512], ps0[:])
                    nc.vector.tensor_copy(fo[:, 512:1024], ps1[:])
                    nc.scalar.dma_start(partial_d[cc * 128 : (cc + 1) * 128, :], fo[:])
                rs_d = dramp.tile([128, D], BF16, tag=f"rs{g}")
                nc.gpsimd.collective_compute(
                    "ReduceScatter",
                    mybir.AluOpType.add,
                    replica_groups=[[0, 1, 2, 3], [4, 5, 6, 7]],
                    ins=[partial_d.opt()],
                    outs=[rs_d.opt()],
                )
                # residual: RS-gated cast-DMA on the GpSimd queue (ordered
                # behind this RS), add on DVE, store on ACT
                rf = rfp.tile([128, D], F32, tag="rf")
                nc.gpsimd.dma_start(rf[:], rs_d[:])
                nc.vector.tensor_add(xrs[g][:], xrs[g][:], rf[:])
                nc.scalar.dma_start(out[g * 128 : (g + 1) * 128, :], xrs[g][:])


def make_in_maps(x, Wk, bk, Wv, bv, Wf, bf):
    """Host-side sharding: returns the per-core input dict list."""
    x = np.asarray(x, np.float32)
    Wk = np.asarray(Wk, np.float32)
    Wv = np.asarray(Wv, np.float32)
    Wf = np.asarray(Wf, np.float32)
    bk = np.asarray(bk, np.float32)
    bv = np.asarray(bv, np.float32)
    bf = np.asarray(bf, np.float32)
    mask = np.tril(np.ones((128, 128), np.float32)).T  # mask[k,q]=1 iff k<=q
    in_maps = []
    for c in range(NCORES):
        b, r = c // GROUP, c % GROUP
        xb = x[b]                                    # [S, D]
        xT = np.ascontiguousarray(xb.T).astype(bf16)
        qTs = xT[DKS * r : DKS * (r + 1)]            # heads 4r..4r+3 rows
        # chunked RS: core (b,r) tile g holds x rows 512g+128r+[0,128)
        xres = np.concatenate(
            [xb[512 * g + 128 * r : 512 * g + 128 * r + 128] for g in range(4)]
        ) + bf[None, :].astype(np.float32)
        in_maps.append({
            "xT": xT,
            "qT": np.ascontiguousarray(qTs),
            "xres": np.ascontiguousarray(xres),
            "wk": np.ascontiguousarray(Wk[:, DKS * r : DKS * (r + 1)]).astype(bf16),
            "wv": np.ascontiguousarray(Wv[:, DVS * r : DVS * (r + 1)]).astype(bf16),
            "wf": np.ascontiguousarray(Wf[DVS * r : DVS * (r + 1), :]).astype(bf16),
            "bkb": bk[None, DKS * r : DKS * (r + 1)].astype(bf16),
            "bvb": bv[None, DVS * r : DVS * (r + 1)].astype(bf16),
            "maskt": mask.astype(bf16),
            "ident": np.eye(128, dtype=np.float32).astype(bf16),
            "onesr": np.ones((1, 512), bf16),
        })
    return in_maps


def assemble(results):
    """[8 x [512,1024]] core outputs -> [2,2048,1024]."""
    out = np.empty((B, S, D), np.float32)
    for c in range(NCORES):
        b, r = c // GROUP, c % GROUP
        for g in range(4):
            out[b, 512 * g + 128 * r : 512 * g + 128 * r + 128, :] = results[c][
                "out"
            ][128 * g : 128 * (g + 1)]
    return out


def kernel(x, Wk, bk, Wv, bv, Wf, bf, _trace=False, _trace_cores=None):
    global _compiled
    if _compiled is None:
        _compiled = build_program()
    nc = _compiled
    in_maps = make_in_maps(x, Wk, bk, Wv, bv, Wf, bf)
    res = bass_utils.run_bass_kernel_spmd(
        nc,
        in_maps,
        core_ids=list(range(NCORES)),
        trace=_trace,
        trace_cores=_trace_cores,
    )
    out = assemble(res.results)
    kernel.last_result = res
    return out



# revision 4
# speedup vs baseline: 1.0458x; 1.0458x over previous
"""Trainium2 Bass kernel for nn_ExperimentalLayer9 (dense transformer layer).

Layer: x + gelu(attn(x)) @ Wf with
  Q = split_heads(x), K = split_heads(x@Wk+bk), V = split_heads(x@Wv+bv)
  causal softmax (no 1/sqrt(d) scale), exact-erf gelu, residual add.

Sharding over 8 NeuronCores: 2 batch groups x 4-way head/tensor parallel.
Core c handles batch b=c//4 and heads [4r, 4r+4) with r=c%4.

v2 pipeline (vs the v1 baseline):
  * q is processed in four 512-row chunks (qc).  Per chunk: scores (row-
    tiled head pairs, interleaved with just-in-time V projection tiles),
    exp on ACT, flash-style AV with a ones-column for the softmax
    normalizer, transpose of o via xbar DMA, gelu -> fp8 on ACT.
  * The FF partial runs in fp8e4m3 with MatmulPerfMode.DoubleRow (two
    128-deep contraction chunks per instruction at 0.5 cycles/row), with
    Wf pre-scaled by 64 on the host and the product rescaled by 1/64 on
    the PSUM->SBUF copy.  FF group g and its 4-rank ReduceScatter are
    issued as soon as q rows [512g, 512g+512) clear attention, so the
    collective overlaps attention of later chunks instead of trailing
    the whole kernel.
  * K projection is chunk-major so matmuls start while xT is still
    streaming from HBM.
"""

import numpy as np
import ml_dtypes

import concourse.bass as bass
import concourse.mybir as mybir
import concourse.tile as tile
from concourse import bacc
from concourse import bass_utils

# Problem shapes (hardcoded per contest contract).
B, S, D, H, DHID = 2, 2048, 1024, 16, 4096
NCORES = 8
GROUP = 4              # cores per batch group
HPC = 4                # heads per core
DK = 64                # q/k head dim
DV = 256               # v head dim
DKS = HPC * DK         # 256  k-slice per core
DVS = HPC * DV         # 1024 v/hidden slice per core
ROWS = S // GROUP      # 512  output rows per core after ReduceScatter
NM = D // 128          # 8    contraction chunks over d_model
VSTRIDE = DV + 1       # 257  V columns per head incl. ones column
NQC = 4                # 512-wide q chunks
WSCALE = 64.0          # host pre-scale on Wf (and Wv when V_FP8) for fp8
V_FP8 = False          # V projection in fp8 DoubleRow (rel-err budget flag)

BF16 = mybir.dt.bfloat16
F32 = mybir.dt.float32
F8 = mybir.dt.float8e4
AF = mybir.ActivationFunctionType
DR = mybir.MatmulPerfMode.DoubleRow

bf16 = ml_dtypes.bfloat16
f8e4 = ml_dtypes.float8_e4m3

_compiled = None


def build_program():
    nc = bacc.Bacc(
        "TRN2",
        target_bir_lowering=False,
        debug=False,
        enable_asserts=True,
        num_devices=NCORES,
    )

    # Per-core inputs (values differ per core; program is SPMD-identical).
    xT = nc.dram_tensor("xT", [D, S], BF16, kind="ExternalInput").ap()
    qT = nc.dram_tensor("qT", [DKS, S], BF16, kind="ExternalInput").ap()
    xres = nc.dram_tensor("xres", [ROWS, D], F32, kind="ExternalInput").ap()
    wk = nc.dram_tensor("wk", [D, DKS], BF16, kind="ExternalInput").ap()
    wf8 = nc.dram_tensor("wf8", [DVS, D], F8, kind="ExternalInput").ap()
    bkb = nc.dram_tensor("bkb", [1, DKS], BF16, kind="ExternalInput").ap()
    bvb = nc.dram_tensor("bvb", [1, DVS], BF16, kind="ExternalInput").ap()
    maskt = nc.dram_tensor("maskt", [128, 128], BF16, kind="ExternalInput").ap()
    onesr = nc.dram_tensor("onesr", [1, 512], BF16, kind="ExternalInput").ap()
    if V_FP8:
        x8T = nc.dram_tensor("x8T", [D, S], F8, kind="ExternalInput").ap()
        wv = nc.dram_tensor("wv8", [D, DVS], F8, kind="ExternalInput").ap()
    else:
        x8T = None
        wv = nc.dram_tensor("wv", [D, DVS], BF16, kind="ExternalInput").ap()
    out = nc.dram_tensor("out", [ROWS, D], F32, kind="ExternalOutput").ap()

    with tile.TileContext(nc) as tc:
        _body(nc, tc, xT, x8T, qT, xres, wk, wv, wf8, bkb, bvb, maskt, onesr, out)

    nc.compile()
    return nc


def _body(nc, tc, xT, x8T, qT, xres, wk, wv, wf8, bkb, bvb, maskt, onesr, out):
    NST = S // 128     # 16 k tiles of 128

    with (
        tc.tile_pool(name="const", bufs=1) as constp,
        tc.tile_pool(name="kv", bufs=1) as kvp,
        tc.tile_pool(name="xt", bufs=1) as xtp,
        tc.tile_pool(name="wts", bufs=1) as wtsp,
        tc.tile_pool(name="exp", bufs=1) as expp,
        tc.tile_pool(name="gotb", bufs=2) as gotbp,
        tc.tile_pool(name="gotf", bufs=2) as gotfp,
        tc.tile_pool(name="ot", bufs=3) as otp,
        tc.tile_pool(name="fo", bufs=2) as fop,
        tc.tile_pool(name="res", bufs=1) as resp,
        tc.tile_pool(name="rf", bufs=2) as rfp,
        tc.tile_pool(name="small", bufs=8) as smallp,
        tc.tile_pool(name="dram", bufs=1, space="DRAM") as dramp,
    ):
        # ---- constants (ACT queue) ------------------------------------
        ones_sb = constp.tile([1, 512], BF16)
        nc.scalar.dma_start(ones_sb[:], onesr[:])
        mask_sb = constp.tile([128, 128], BF16)
        nc.scalar.dma_start(mask_sb[:], maskt[:])
        bk_sb = constp.tile([1, DKS], BF16)
        nc.scalar.dma_start(bk_sb[:], bkb[:])
        bv_sb = constp.tile([1, DVS], BF16)
        nc.scalar.dma_start(bv_sb[:], bvb[:])

        # Warm up the collectives path (ncfw/channel setup) so the first
        # real ReduceScatter doesn't pay ~25us of first-call overhead.
        warm_in = dramp.tile([4, 16], BF16, tag="warm_in")
        warm_out = dramp.tile([1, 16], BF16, tag="warm_out")
        nc.scalar.dma_start(
            warm_in[:].rearrange("a b -> (a b)")[None, :], ones_sb[0:1, 0:64]
        )
        nc.gpsimd.collective_compute(
            "ReduceScatter",
            mybir.AluOpType.add,
            replica_groups=[[0, 1, 2, 3], [4, 5, 6, 7]],
            ins=[warm_in.opt()],
            outs=[warm_out.opt()],
        )

        # ---- bulk input loads -----------------------------------------
        # xT chunk m lands right before the K-proj matmuls that need it
        # (Sync queue, chunk-major consumer below).
        xT_sb = xtp.tile([128, NM * S], BF16, tag="xT")
        for m in range(NM):
            nc.sync.dma_start(
                xT_sb[:, m * S : (m + 1) * S], xT[m * 128 : (m + 1) * 128, :]
            )
        qT_sb = kvp.tile([128, 2 * S], BF16)
        for m in range(2):
            nc.sync.dma_start(
                qT_sb[:, m * S : (m + 1) * S], qT[m * 128 : (m + 1) * 128, :]
            )
        wk_sb = wtsp.tile([128, NM * DKS], BF16, tag="wk")
        for m in range(NM):
            nc.scalar.dma_start(
                wk_sb[:, m * DKS : (m + 1) * DKS], wk[m * 128 : (m + 1) * 128, :]
            )
        wv_sb = wtsp.tile([128, NM * DVS], wv.dtype, tag="wv")
        for m in range(NM):
            nc.scalar.dma_start(
                wv_sb[:, m * DVS : (m + 1) * DVS], wv[m * 128 : (m + 1) * 128, :]
            )
        wf8_sb = wtsp.tile([128, NM * D], F8, tag="wf8")
        for m in range(NM):
            nc.scalar.dma_start(
                wf8_sb[:, m * D : (m + 1) * D], wf8[m * 128 : (m + 1) * 128, :]
            )
        if V_FP8:
            x8T_sb = xtp.tile([128, NM * S], F8, tag="x8T")
            for m in range(NM):
                nc.sync.dma_start(
                    x8T_sb[:, m * S : (m + 1) * S], x8T[m * 128 : (m + 1) * 128, :]
                )
        # residual rows (gpsimd queue; only needed at RS time)
        xres_sb = resp.tile([128, 4 * D], F32)
        for g in range(4):
            nc.gpsimd.dma_start(
                xres_sb[:, g * D : (g + 1) * D], xres[g * 128 : (g + 1) * 128, :]
            )

        kt_sb = kvp.tile([128, 2 * S], BF16)   # K^T rows dk%128, chunk dk//128
        v_sb = kvp.tile([128, NST * HPC * VSTRIDE], BF16)
        # softmax ones-columns (written once; V-proj copies skip them)
        nc.vector.memset(
            v_sb[:].rearrange("p (t h c) -> p t h c", t=NST, h=HPC)[:, :, :, DV],
            1.0,
        )

        # ---- K^T projection: chunk-major, overlaps the xT stream ------
        with tc.tile_pool(name="psK", bufs=4, space="PSUM") as psK:
            for dkt in range(2):
                pss = []
                for st in range(4):
                    ps = psK.tile([128, 512], F32, tag="k")
                    nc.tensor.matmul(
                        ps[:],
                        bk_sb[:, dkt * 128 : (dkt + 1) * 128],
                        ones_sb[:, 0:512],
                        start=True,
                        stop=False,
                    )
                    pss.append(ps)
                for m in range(NM):
                    for st in range(4):
                        nc.tensor.matmul(
                            pss[st][:],
                            wk_sb[:, m * DKS + dkt * 128 : m * DKS + dkt * 128 + 128],
                            xT_sb[:, m * S + st * 512 : m * S + st * 512 + 512],
                            start=False,
                            stop=(m == NM - 1),
                        )
                for st in range(4):
                    nc.scalar.copy(
                        kt_sb[:, dkt * S + st * 512 : dkt * S + st * 512 + 512],
                        pss[st][:],
                    )

        # ---- main pipeline: scores/V-proj/AV per 512-q chunk, FF+RS ---
        with (
            tc.tile_pool(name="psV", bufs=2, space="PSUM") as psV,
            tc.tile_pool(name="psS", bufs=2, space="PSUM") as psS,
            tc.tile_pool(name="psA", bufs=2, space="PSUM") as psA,
            tc.tile_pool(name="psF", bufs=2, space="PSUM") as psF,
        ):
            def v_tile(st, dvh):
                """V[s-tile, 512 dv cols] for heads (2dvh, 2dvh+1)."""
                ps = psV.tile([128, 512], F32, tag="v")
                nc.tensor.matmul(
                    ps[:],
                    ones_sb[:, 0:128],
                    bv_sb[:, dvh * 512 : dvh * 512 + 512],
                    start=True,
                    stop=False,
                )
                if V_FP8:
                    x8r = x8T_sb[:].rearrange("p (m s) -> p m s", m=NM)
                    wvr = wv_sb[:].rearrange("p (m d) -> p m d", m=NM)
                    for mp in range(NM // 2):
                        nc.tensor.matmul(
                            ps[:],
                            x8r[:, 2 * mp : 2 * mp + 2, st * 128 : st * 128 + 128],
                            wvr[:, 2 * mp : 2 * mp + 2, dvh * 512 : dvh * 512 + 512],
                            start=False,
                            stop=(mp == NM // 2 - 1),
                            perf_mode=DR,
                        )
                else:
                    for m in range(NM):
                        nc.tensor.matmul(
                            ps[:],
                            xT_sb[:, m * S + st * 128 : m * S + st * 128 + 128],
                            wv_sb[:, m * DVS + dvh * 512 : m * DVS + dvh * 512 + 512],
                            start=False,
                            stop=(m == NM - 1),
                        )
                base = st * HPC * VSTRIDE
                for hh in range(2):
                    h = 2 * dvh + hh
                    dst = v_sb[:, base + h * VSTRIDE : base + h * VSTRIDE + DV]
                    if V_FP8:
                        nc.scalar.activation(
                            dst, ps[:, hh * 256 : hh * 256 + 256],
                            AF.Identity, scale=1.0 / WSCALE,
                        )
                    else:
                        nc.scalar.copy(dst, ps[:, hh * 256 : hh * 256 + 256])

            def sc_tile(pair, qc, kt, hl, exps_t):
                """scores^T[k-tile kt, q chunk qc] for head 2*pair+hl."""
                po = 64 * hl
                t = kt - 4 * qc
                toff = max(t, 0) * 128
                ps = psS.tile([128, 512], F32, tag="s")
                nc.tensor.matmul(
                    ps[:, toff:512],
                    kt_sb[po : po + 64, pair * S + kt * 128 : pair * S + kt * 128 + 128],
                    qT_sb[po : po + 64,
                          pair * S + qc * 512 + toff : pair * S + qc * 512 + 512],
                    start=True,
                    stop=True,
                    tile_position=(po, 0),
                )
                nc.scalar.activation(
                    exps_t[:, kt * 512 + toff : (kt + 1) * 512], ps[:, toff:512],
                    AF.Exp,
                )
                if t >= 0:  # mask the diagonal 128x128 block
                    blk = exps_t[:, kt * 512 + toff : kt * 512 + toff + 128]
                    nc.vector.tensor_mul(blk, blk, mask_sb[:])

            def av_tile(head, qc, sq, exps_t, gotb):
                """o[q-tile sq, dv] for head; transpose into gotb columns."""
                pso = psA.tile([128, VSTRIDE], F32, tag="a")
                for kt in range(sq + 1):
                    vb = kt * HPC * VSTRIDE + head * VSTRIDE
                    nc.tensor.matmul(
                        pso[:],
                        exps_t[:, kt * 512 + (sq - 4 * qc) * 128
                               : kt * 512 + (sq - 4 * qc) * 128 + 128],
                        v_sb[:, vb : vb + VSTRIDE],
                        start=(kt == 0),
                        stop=(kt == sq),
                    )
                recip = smallp.tile([128, 1], F32, tag="recip")
                nc.vector.reciprocal(recip[:], pso[:, DV : DV + 1])
                ot = otp.tile([128, DV], BF16, tag="ot")
                nc.vector.tensor_scalar_mul(ot[:], pso[:, 0:DV], recip[:])
                qo = (sq - 4 * qc) * 128
                for half in range(2):
                    hc = 2 * head + half
                    nc.sync.dma_start_transpose(
                        gotb[:, hc * 512 + qo : hc * 512 + qo + 128],
                        ot[:, half * 128 : half * 128 + 128],
                    )

            def ff_group(g, gotf):
                """FF partial for q rows [512g, 512g+512) in fp8 DoubleRow,
                then chunked ReduceScatter + residual + output DMA."""
                gfr = gotf[:].rearrange("p (h q) -> p h q", h=2 * HPC)
                wfr = wf8_sb[:].rearrange("p (h d) -> p h d", h=NM)
                partial_d = dramp.tile([512, D], BF16, tag=f"part{g}")
                for qt in range(4):
                    fo = fop.tile([128, D], BF16, tag="fo")
                    for half in range(2):
                        ps = psF.tile([128, 512], F32, tag="f")
                        for dr in range(4):
                            nc.tensor.matmul(
                                ps[:],
                                gfr[:, 2 * dr : 2 * dr + 2,
                                    qt * 128 : qt * 128 + 128],
                                wfr[:, 2 * dr : 2 * dr + 2,
                                    half * 512 : half * 512 + 512],
                                start=(dr == 0),
                                stop=(dr == 3),
                                perf_mode=DR,
                            )
                        nc.vector.tensor_scalar_mul(
                            fo[:, half * 512 : half * 512 + 512], ps[:],
                            1.0 / WSCALE,
                        )
                    nc.gpsimd.dma_start(
                        partial_d[qt * 128 : (qt + 1) * 128, :], fo[:]
                    )
                rs_d = dramp.tile([128, D], BF16, tag=f"rs{g}")
                nc.gpsimd.collective_compute(
                    "ReduceScatter",
                    mybir.AluOpType.add,
                    replica_groups=[[0, 1, 2, 3], [4, 5, 6, 7]],
                    ins=[partial_d.opt()],
                    outs=[rs_d.opt()],
                )
                # residual: RS-gated cast-DMA on the GpSimd queue (ordered
                # behind this RS), add on DVE, store on ACT
                rf = rfp.tile([128, D], F32, tag="rf")
                nc.gpsimd.dma_start(rf[:], rs_d[:])
                nc.vector.tensor_add(rf[:], rf[:], xres_sb[:, g * D : (g + 1) * D])
                nc.scalar.dma_start(out[g * 128 : (g + 1) * 128, :], rf[:])

            # ---- the pipeline ----
            # Two exp tiles (one per head of the active pair); pair 1 of a
            # chunk reuses them after pair 0's AV has drained.
            exps = {}
            for hl in range(2):
                exps_t = expp.tile([128, NST * 512], BF16, tag=f"e{hl}")
                exps[hl] = exps_t
            gotbs, gotfs = [], []
            for qc in range(NQC):
                # FF of the previous group (its transposes/gelu are done by
                # now) leads the chunk
                if qc >= 1:
                    ff_group(qc - 1, gotfs[qc - 1])
                gotb = gotbp.tile([128, 2 * HPC * 512], BF16, tag="gotb")
                nkt = 4 * qc + 4
                for pair in range(2):
                    # scores of this pair interleaved with the JIT V tiles
                    # its AV needs (dvh == pair covers heads 2p, 2p+1)
                    vts = [(4 * qc + i, pair) for i in range(4)]
                    vi = 0
                    for kt in range(nkt):
                        sc_tile(pair, qc, kt, 0, exps[0])
                        sc_tile(pair, qc, kt, 1, exps[1])
                        while vi * nkt < (kt + 1) * len(vts):
                            v_tile(*vts[vi])
                            vi += 1
                    for hl in range(2):
                        for sq in range(4 * qc, 4 * qc + 4):
                            av_tile(2 * pair + hl, qc, sq, exps[hl], gotb)
                # gelu (exact erf) -> fp8 per hidden chunk of this group
                gotf = gotfp.tile([128, 2 * HPC * 512], F8, tag="gotf")
                for hc in range(2 * HPC):
                    nc.scalar.activation(
                        gotf[:, hc * 512 : (hc + 1) * 512],
                        gotb[:, hc * 512 : (hc + 1) * 512],
                        AF.Gelu,
                    )
                gotbs.append(gotb)
                gotfs.append(gotf)
            ff_group(NQC - 1, gotfs[NQC - 1])


def make_in_maps(x, Wk, bk, Wv, bv, Wf, bf):
    """Host-side sharding: returns the per-core input dict list."""
    x = np.asarray(x, np.float32)
    Wk = np.asarray(Wk, np.float32)
    Wv = np.asarray(Wv, np.float32)
    Wf = np.asarray(Wf, np.float32)
    bk = np.asarray(bk, np.float32)
    bv = np.asarray(bv, np.float32)
    bf = np.asarray(bf, np.float32)
    mask = np.tril(np.ones((128, 128), np.float32)).T  # mask[k,q]=1 iff k<=q
    in_maps = []
    for c in range(NCORES):
        b, r = c // GROUP, c % GROUP
        xb = x[b]                                    # [S, D]
        xT = np.ascontiguousarray(xb.T).astype(bf16)
        qTs = xT[DKS * r : DKS * (r + 1)]            # heads 4r..4r+3 rows
        # chunked RS: core (b,r) tile g holds x rows 512g+128r+[0,128)
        xres = np.concatenate(
            [xb[512 * g + 128 * r : 512 * g + 128 * r + 128] for g in range(4)]
        ) + bf[None, :].astype(np.float32)
        m = {
            "xT": xT,
            "qT": np.ascontiguousarray(qTs),
            "xres": np.ascontiguousarray(xres),
            "wk": np.ascontiguousarray(Wk[:, DKS * r : DKS * (r + 1)]).astype(bf16),
            "wf8": np.ascontiguousarray(
                Wf[DVS * r : DVS * (r + 1), :] * WSCALE
            ).astype(f8e4),
            "bkb": bk[None, DKS * r : DKS * (r + 1)].astype(bf16),
            "maskt": mask.astype(bf16),
            "onesr": np.ones((1, 512), bf16),
        }
        wvs = np.ascontiguousarray(Wv[:, DVS * r : DVS * (r + 1)])
        if V_FP8:
            m["x8T"] = xT.astype(f8e4)
            m["wv8"] = (wvs * WSCALE).astype(f8e4)
            m["bvb"] = (bv[None, DVS * r : DVS * (r + 1)] * WSCALE).astype(bf16)
        else:
            m["wv"] = wvs.astype(bf16)
            m["bvb"] = bv[None, DVS * r : DVS * (r + 1)].astype(bf16)
        in_maps.append(m)
    return in_maps


def assemble(results):
    """[8 x [512,1024]] core outputs -> [2,2048,1024]."""
    out = np.empty((B, S, D), np.float32)
    for c in range(NCORES):
        b, r = c // GROUP, c % GROUP
        for g in range(4):
            out[b, 512 * g + 128 * r : 512 * g + 128 * r + 128, :] = results[c][
                "out"
            ][128 * g : 128 * (g + 1)]
    return out


def kernel(x, Wk, bk, Wv, bv, Wf, bf, _trace=False, _trace_cores=None):
    global _compiled
    if _compiled is None:
        _compiled = build_program()
    nc = _compiled
    in_maps = make_in_maps(x, Wk, bk, Wv, bv, Wf, bf)
    res = bass_utils.run_bass_kernel_spmd(
        nc,
        in_maps,
        core_ids=list(range(NCORES)),
        trace=_trace,
        trace_cores=_trace_cores,
    )
    out = assemble(res.results)
    kernel.last_result = res
    return out


# revision 9
# speedup vs baseline: 1.1180x; 1.0690x over previous
"""Trainium2 Bass kernel for nn_ExperimentalLayer9 (dense transformer layer).

Layer: x + gelu(attn(x)) @ Wf with
  Q = split_heads(x), K = split_heads(x@Wk+bk), V = split_heads(x@Wv+bv)
  causal softmax (no 1/sqrt(d) scale), exact-erf gelu, residual add.

Sharding over 8 NeuronCores: 2 batch groups x 4-way head/tensor parallel.
Core c handles batch b=c//4 and heads [4r, 4r+4) with r=c%4.

v2 pipeline (vs the v1 baseline):
  * q is processed in four 512-row chunks (qc).  Per chunk: scores (row-
    tiled head pairs, interleaved with just-in-time V projection tiles),
    exp on ACT, flash-style AV with a ones-column for the softmax
    normalizer, transpose of o via xbar DMA, gelu -> fp8 on ACT.
  * The FF partial runs in fp8e4m3 with MatmulPerfMode.DoubleRow (two
    128-deep contraction chunks per instruction at 0.5 cycles/row), with
    Wf pre-scaled by 64 on the host and the product rescaled by 1/64 on
    the PSUM->SBUF copy.  FF group g and its 4-rank ReduceScatter are
    issued as soon as q rows [512g, 512g+512) clear attention, so the
    collective overlaps attention of later chunks instead of trailing
    the whole kernel.
  * K projection is chunk-major so matmuls start while xT is still
    streaming from HBM.
"""

import numpy as np
import ml_dtypes

import concourse.bass as bass
import concourse.mybir as mybir
import concourse.tile as tile
from concourse import bacc
from concourse import bass_utils

# Problem shapes (hardcoded per contest contract).
B, S, D, H, DHID = 2, 2048, 1024, 16, 4096
NCORES = 8
GROUP = 4              # cores per batch group
HPC = 4                # heads per core
DK = 64                # q/k head dim
DV = 256               # v head dim
DKS = HPC * DK         # 256  k-slice per core
DVS = HPC * DV         # 1024 v/hidden slice per core
ROWS = S // GROUP      # 512  output rows per core after ReduceScatter
NM = D // 128          # 8    contraction chunks over d_model
VSTRIDE = DV + 1       # 257  V columns per head incl. ones column
NQC = 4                # 512-wide q chunks
WSCALE = 64.0          # host pre-scale on Wf (and Wv when V_FP8) for fp8
V_FP8 = False          # V projection in fp8 DoubleRow (rel-err budget flag)

BF16 = mybir.dt.bfloat16
F32 = mybir.dt.float32
F8 = mybir.dt.float8e4
AF = mybir.ActivationFunctionType
DR = mybir.MatmulPerfMode.DoubleRow

bf16 = ml_dtypes.bfloat16
f8e4 = ml_dtypes.float8_e4m3

_compiled = None


def build_program():
    nc = bacc.Bacc(
        "TRN2",
        target_bir_lowering=False,
        debug=False,
        enable_asserts=True,
        num_devices=NCORES,
    )

    # Per-core inputs (values differ per core; program is SPMD-identical).
    xT = nc.dram_tensor("xT", [D, S], BF16, kind="ExternalInput").ap()
    qT = nc.dram_tensor("qT", [DKS, S], BF16, kind="ExternalInput").ap()
    xres = nc.dram_tensor("xres", [ROWS, D], F32, kind="ExternalInput").ap()
    wk = nc.dram_tensor("wk", [D, DKS], BF16, kind="ExternalInput").ap()
    wf8 = nc.dram_tensor("wf8", [DVS, D], F8, kind="ExternalInput").ap()
    bkb = nc.dram_tensor("bkb", [1, DKS], BF16, kind="ExternalInput").ap()
    bvb = nc.dram_tensor("bvb", [1, DVS], BF16, kind="ExternalInput").ap()
    maskt = nc.dram_tensor("maskt", [128, 128], BF16, kind="ExternalInput").ap()
    onesr = nc.dram_tensor("onesr", [1, 512], BF16, kind="ExternalInput").ap()
    if V_FP8:
        x8T = nc.dram_tensor("x8T", [D, S], F8, kind="ExternalInput").ap()
        wv = nc.dram_tensor("wv8", [D, DVS], F8, kind="ExternalInput").ap()
    else:
        x8T = None
        wv = nc.dram_tensor("wv", [D, DVS], BF16, kind="ExternalInput").ap()
    out = nc.dram_tensor("out", [ROWS, D], F32, kind="ExternalOutput").ap()

    with tile.TileContext(nc) as tc:
        _body(nc, tc, xT, x8T, qT, xres, wk, wv, wf8, bkb, bvb, maskt, onesr, out)

    nc.compile()
    return nc


def _body(nc, tc, xT, x8T, qT, xres, wk, wv, wf8, bkb, bvb, maskt, onesr, out):
    NST = S // 128     # 16 k tiles of 128

    with (
        tc.tile_pool(name="const", bufs=1) as constp,
        tc.tile_pool(name="kv", bufs=1) as kvp,
        tc.tile_pool(name="xt", bufs=1) as xtp,
        tc.tile_pool(name="wts", bufs=1) as wtsp,
        tc.tile_pool(name="exp", bufs=1) as expp,
        tc.tile_pool(name="gotb", bufs=2) as gotbp,
        tc.tile_pool(name="gotf", bufs=2) as gotfp,
        tc.tile_pool(name="ot", bufs=3) as otp,
        tc.tile_pool(name="fo", bufs=2) as fop,
        tc.tile_pool(name="res", bufs=1) as resp,
        tc.tile_pool(name="rf", bufs=2) as rfp,
        tc.tile_pool(name="small", bufs=8) as smallp,
        tc.tile_pool(name="dram", bufs=1, space="DRAM") as dramp,
    ):
        # ---- constants (ACT queue) ------------------------------------
        ones_sb = constp.tile([1, 512], BF16)
        nc.scalar.dma_start(ones_sb[:], onesr[:])
        mask_sb = constp.tile([128, 128], BF16)
        nc.scalar.dma_start(mask_sb[:], maskt[:])
        bk_sb = constp.tile([1, DKS], BF16)
        nc.scalar.dma_start(bk_sb[:], bkb[:])
        bv_sb = constp.tile([1, DVS], BF16)
        nc.scalar.dma_start(bv_sb[:], bvb[:])

        # Warm up the collectives path (ncfw/channel setup) so the first
        # real ReduceScatter doesn't pay ~25us of first-call overhead.
        warm_in = dramp.tile([4, 16], BF16, tag="warm_in")
        warm_out = dramp.tile([1, 16], BF16, tag="warm_out")
        nc.scalar.dma_start(
            warm_in[:].rearrange("a b -> (a b)")[None, :], ones_sb[0:1, 0:64]
        )
        nc.gpsimd.collective_compute(
            "ReduceScatter",
            mybir.AluOpType.add,
            replica_groups=[[0, 1, 2, 3], [4, 5, 6, 7]],
            ins=[warm_in.opt()],
            outs=[warm_out.opt()],
        )

        # ---- bulk input loads -----------------------------------------
        # xT chunk m lands right before the K-proj matmuls that need it
        # (Sync queue, chunk-major consumer below).
        xT_sb = xtp.tile([128, NM * S], BF16, tag="xT")
        for m in range(NM):
            nc.sync.dma_start(
                xT_sb[:, m * S : (m + 1) * S], xT[m * 128 : (m + 1) * 128, :]
            )
        qT_sb = kvp.tile([128, 2 * S], BF16)
        for m in range(2):
            nc.sync.dma_start(
                qT_sb[:, m * S : (m + 1) * S], qT[m * 128 : (m + 1) * 128, :]
            )
        wk_sb = wtsp.tile([128, NM * DKS], BF16, tag="wk")
        for m in range(NM):
            nc.scalar.dma_start(
                wk_sb[:, m * DKS : (m + 1) * DKS], wk[m * 128 : (m + 1) * 128, :]
            )
        wv_sb = wtsp.tile([128, NM * DVS], wv.dtype, tag="wv")
        for m in range(NM):
            nc.scalar.dma_start(
                wv_sb[:, m * DVS : (m + 1) * DVS], wv[m * 128 : (m + 1) * 128, :]
            )
        wf8_sb = wtsp.tile([128, NM * D], F8, tag="wf8")
        for m in range(NM):
            nc.scalar.dma_start(
                wf8_sb[:, m * D : (m + 1) * D], wf8[m * 128 : (m + 1) * 128, :]
            )
        if V_FP8:
            x8T_sb = xtp.tile([128, NM * S], F8, tag="x8T")
            for m in range(NM):
                nc.sync.dma_start(
                    x8T_sb[:, m * S : (m + 1) * S], x8T[m * 128 : (m + 1) * 128, :]
                )
        # residual rows (gpsimd queue; only needed at RS time)
        xres_sb = resp.tile([128, 4 * D], F32)
        for g in range(4):
            nc.gpsimd.dma_start(
                xres_sb[:, g * D : (g + 1) * D], xres[g * 128 : (g + 1) * 128, :]
            )

        kt_sb = kvp.tile([128, 2 * S], BF16)   # K^T rows dk%128, chunk dk//128
        v_sb = kvp.tile([128, NST * HPC * VSTRIDE], BF16)
        # softmax ones-columns (written once; V-proj copies skip them)
        nc.vector.memset(
            v_sb[:].rearrange("p (t h c) -> p t h c", t=NST, h=HPC)[:, :, :, DV],
            1.0,
        )

        # ---- K^T projection: chunk-major, overlaps the xT stream ------
        with tc.tile_pool(name="psK", bufs=4, space="PSUM") as psK:
            for dkt in range(2):
                pss = []
                for st in range(4):
                    ps = psK.tile([128, 512], F32, tag="k")
                    nc.tensor.matmul(
                        ps[:],
                        bk_sb[:, dkt * 128 : (dkt + 1) * 128],
                        ones_sb[:, 0:512],
                        start=True,
                        stop=False,
                    )
                    pss.append(ps)
                for m in range(NM):
                    for st in range(4):
                        nc.tensor.matmul(
                            pss[st][:],
                            wk_sb[:, m * DKS + dkt * 128 : m * DKS + dkt * 128 + 128],
                            xT_sb[:, m * S + st * 512 : m * S + st * 512 + 512],
                            start=False,
                            stop=(m == NM - 1),
                        )
                for st in range(4):
                    nc.vector.tensor_copy(
                        kt_sb[:, dkt * S + st * 512 : dkt * S + st * 512 + 512],
                        pss[st][:],
                    )

        # ---- main pipeline: scores/V-proj/AV per 512-q chunk, FF+RS ---
        with (
            tc.tile_pool(name="psV", bufs=1, space="PSUM") as psV,
            tc.tile_pool(name="psS", bufs=2, space="PSUM") as psS,
            tc.tile_pool(name="psA", bufs=2, space="PSUM") as psA,
            tc.tile_pool(name="psF", bufs=1, space="PSUM") as psF,
        ):
            def v_tile(st, dvh):
                """V[s-tile, 512 dv cols] for heads (2dvh, 2dvh+1)."""
                ps = psV.tile([128, 512], F32, tag="v")
                nc.tensor.matmul(
                    ps[:],
                    ones_sb[:, 0:128],
                    bv_sb[:, dvh * 512 : dvh * 512 + 512],
                    start=True,
                    stop=False,
                )
                if V_FP8:
                    x8r = x8T_sb[:].rearrange("p (m s) -> p m s", m=NM)
                    wvr = wv_sb[:].rearrange("p (m d) -> p m d", m=NM)
                    for mp in range(NM // 2):
                        nc.tensor.matmul(
                            ps[:],
                            x8r[:, 2 * mp : 2 * mp + 2, st * 128 : st * 128 + 128],
                            wvr[:, 2 * mp : 2 * mp + 2, dvh * 512 : dvh * 512 + 512],
                            start=False,
                            stop=(mp == NM // 2 - 1),
                            perf_mode=DR,
                        )
                else:
                    for m in range(NM):
                        nc.tensor.matmul(
                            ps[:],
                            xT_sb[:, m * S + st * 128 : m * S + st * 128 + 128],
                            wv_sb[:, m * DVS + dvh * 512 : m * DVS + dvh * 512 + 512],
                            start=False,
                            stop=(m == NM - 1),
                        )
                base = st * HPC * VSTRIDE
                for hh in range(2):
                    h = 2 * dvh + hh
                    dst = v_sb[:, base + h * VSTRIDE : base + h * VSTRIDE + DV]
                    if V_FP8:
                        nc.scalar.activation(
                            dst, ps[:, hh * 256 : hh * 256 + 256],
                            AF.Identity, scale=1.0 / WSCALE,
                        )
                    else:
                        nc.vector.tensor_copy(dst, ps[:, hh * 256 : hh * 256 + 256])

            def sc_tile(pair, qc, kt, exps_t):
                """scores^T[k-tile kt, q chunk qc] for heads 2p, 2p+1.
                Both heads run concurrently in PE row quadrants and share one
                [128,1024] PSUM tile (head hl at columns hl*512+...), drained
                by a single 2-segment Exp activation."""
                t = kt - 4 * qc
                toff = max(t, 0) * 128
                w = 512 - toff
                ps = psS.tile([128, 1024], F32, tag="s")
                for hl in range(2):
                    po = 64 * hl
                    nc.tensor.matmul(
                        ps[:, hl * 512 + toff : hl * 512 + 512],
                        kt_sb[po : po + 64,
                              pair * S + kt * 128 : pair * S + kt * 128 + 128],
                        qT_sb[po : po + 64,
                              pair * S + qc * 512 + toff : pair * S + qc * 512 + 512],
                        start=True,
                        stop=True,
                        tile_position=(po, 0),
                    )
                psr = ps[:].rearrange("p (h w) -> p h w", h=2)
                er = exps_t[:].rearrange("p (t h w) -> p t h w", t=NST, h=2)
                nc.scalar.activation(
                    er[:, kt, :, toff:512], psr[:, :, toff:512], AF.Exp
                )
                if t >= 0:  # mask the diagonal 128x128 block of both heads
                    for hl in range(2):
                        blk = exps_t[:, kt * 1024 + hl * 512 + toff
                                     : kt * 1024 + hl * 512 + toff + 128]
                        nc.vector.tensor_mul(blk, blk, mask_sb[:])

            def av_tile(head, qc, sq, exps_t, gotb):
                """o[q-tile sq, dv] for head; transpose into gotb columns."""
                hl = head % 2
                pso = psA.tile([128, VSTRIDE], F32, tag="a")
                for kt in range(sq + 1):
                    vb = kt * HPC * VSTRIDE + head * VSTRIDE
                    eo = kt * 1024 + hl * 512 + (sq - 4 * qc) * 128
                    nc.tensor.matmul(
                        pso[:],
                        exps_t[:, eo : eo + 128],
                        v_sb[:, vb : vb + VSTRIDE],
                        start=(kt == 0),
                        stop=(kt == sq),
                    )
                recip = smallp.tile([128, 1], F32, tag="recip")
                nc.vector.reciprocal(recip[:], pso[:, DV : DV + 1])
                ot = otp.tile([128, DV], BF16, tag="ot")
                nc.vector.tensor_scalar_mul(ot[:], pso[:, 0:DV], recip[:])
                qo = (sq - 4 * qc) * 128
                for half in range(2):
                    hc = 2 * head + half
                    nc.sync.dma_start_transpose(
                        gotb[:, hc * 512 + qo : hc * 512 + qo + 128],
                        ot[:, half * 128 : half * 128 + 128],
                    )

            def ff_group(g, gotf):
                """FF partial for q rows [512g, 512g+512) in fp8 DoubleRow,
                then chunked ReduceScatter + residual + output DMA."""
                gfr = gotf[:].rearrange("p (h q) -> p h q", h=2 * HPC)
                wfr = wf8_sb[:].rearrange("p (h d) -> p h d", h=NM)
                partial_d = dramp.tile([512, D], BF16, tag=f"part{g}")
                for qt in range(4):
                    fo = fop.tile([128, D], BF16, tag="fo")
                    for half in range(2):
                        ps = psF.tile([128, 512], F32, tag="f")
                        for dr in range(4):
                            nc.tensor.matmul(
                                ps[:],
                                gfr[:, 2 * dr : 2 * dr + 2,
                                    qt * 128 : qt * 128 + 128],
                                wfr[:, 2 * dr : 2 * dr + 2,
                                    half * 512 : half * 512 + 512],
                                start=(dr == 0),
                                stop=(dr == 3),
                                perf_mode=DR,
                            )
                        nc.vector.tensor_scalar_mul(
                            fo[:, half * 512 : half * 512 + 512], ps[:],
                            1.0 / WSCALE,
                        )
                    nc.gpsimd.dma_start(
                        partial_d[qt * 128 : (qt + 1) * 128, :], fo[:]
                    )
                rs_d = dramp.tile([128, D], BF16, tag=f"rs{g}")
                nc.gpsimd.collective_compute(
                    "ReduceScatter",
                    mybir.AluOpType.add,
                    replica_groups=[[0, 1, 2, 3], [4, 5, 6, 7]],
                    ins=[partial_d.opt()],
                    outs=[rs_d.opt()],
                )
                # residual: RS-gated cast-DMA on the GpSimd queue (ordered
                # behind this RS), add on DVE, store on ACT
                rf = rfp.tile([128, D], F32, tag="rf")
                nc.gpsimd.dma_start(rf[:], rs_d[:])
                nc.vector.tensor_add(rf[:], rf[:], xres_sb[:, g * D : (g + 1) * D])
                nc.scalar.dma_start(out[g * 128 : (g + 1) * 128, :], rf[:])

            # ---- the pipeline ----
            # One exp tile for the active pair (heads at +-512 within each
            # 1024-wide kt strip); the next pair block reuses it after this
            # pair's AV has drained (guaranteed by PE program order).
            exps_t = expp.tile([128, NST * 1024], BF16, tag="e")
            gotbs, gotfs = [], []
            # gelu strips whose transposes are complete but not yet emitted:
            # list of (gotf, gotb, pair)
            pend_gelu = []

            def flush_gelu():
                for gf, gb, p in pend_gelu:
                    nc.scalar.activation(
                        gf[:, p * 2048 : (p + 1) * 2048],
                        gb[:, p * 2048 : (p + 1) * 2048],
                        AF.Gelu,
                    )
                pend_gelu.clear()

            for qc in range(NQC):
                gotb = gotbp.tile([128, 2 * HPC * 512], BF16, tag="gotb")
                gotf = gotfp.tile([128, 2 * HPC * 512], F8, tag="gotf")
                gotbs.append(gotb)
                gotfs.append(gotf)
                for pair in range(2):
                    # leading scores (kt tiles the first AV needs) with the
                    # JIT V tiles for this pair's heads paced in between;
                    # pending gelu strips are emitted here, while ACT would
                    # otherwise idle
                    vts = [(4 * qc + i, pair) for i in range(4)]
                    vi = 0
                    nlead = 4 * qc + 1
                    for kt in range(nlead):
                        sc_tile(pair, qc, kt, exps_t)
                        while vi * nlead < (kt + 1) * (len(vts) - 1):
                            v_tile(*vts[vi])
                            vi += 1
                    flush_gelu()
                    while vi < len(vts):
                        v_tile(*vts[vi])
                        vi += 1
                    # AV with one-kt-lookahead score production
                    for sq in range(4 * qc, 4 * qc + 4):
                        if sq < 4 * qc + 3:
                            sc_tile(pair, qc, sq + 1, exps_t)
                        av_tile(2 * pair + 0, qc, sq, exps_t, gotb)
                        av_tile(2 * pair + 1, qc, sq, exps_t, gotb)
                    pend_gelu.append((gotf, gotb, pair))
                    # FF of the previous q group goes between this chunk's
                    # pair blocks (its gelu finished during the p0 leading
                    # scores)
                    if pair == 0 and qc >= 1:
                        ff_group(qc - 1, gotfs[qc - 1])
            flush_gelu()
            ff_group(NQC - 1, gotfs[NQC - 1])


def make_in_maps(x, Wk, bk, Wv, bv, Wf, bf):
    """Host-side sharding: returns the per-core input dict list."""
    x = np.asarray(x, np.float32)
    Wk = np.asarray(Wk, np.float32)
    Wv = np.asarray(Wv, np.float32)
    Wf = np.asarray(Wf, np.float32)
    bk = np.asarray(bk, np.float32)
    bv = np.asarray(bv, np.float32)
    bf = np.asarray(bf, np.float32)
    mask = np.tril(np.ones((128, 128), np.float32)).T  # mask[k,q]=1 iff k<=q
    in_maps = []
    for c in range(NCORES):
        b, r = c // GROUP, c % GROUP
        xb = x[b]                                    # [S, D]
        xT = np.ascontiguousarray(xb.T).astype(bf16)
        qTs = xT[DKS * r : DKS * (r + 1)]            # heads 4r..4r+3 rows
        # chunked RS: core (b,r) tile g holds x rows 512g+128r+[0,128)
        xres = np.concatenate(
            [xb[512 * g + 128 * r : 512 * g + 128 * r + 128] for g in range(4)]
        ) + bf[None, :].astype(np.float32)
        m = {
            "xT": xT,
            "qT": np.ascontiguousarray(qTs),
            "xres": np.ascontiguousarray(xres),
            "wk": np.ascontiguousarray(Wk[:, DKS * r : DKS * (r + 1)]).astype(bf16),
            "wf8": np.ascontiguousarray(
                Wf[DVS * r : DVS * (r + 1), :] * WSCALE
            ).astype(f8e4),
            "bkb": bk[None, DKS * r : DKS * (r + 1)].astype(bf16),
            "maskt": mask.astype(bf16),
            "onesr": np.ones((1, 512), bf16),
        }
        wvs = np.ascontiguousarray(Wv[:, DVS * r : DVS * (r + 1)])
        if V_FP8:
            m["x8T"] = xT.astype(f8e4)
            m["wv8"] = (wvs * WSCALE).astype(f8e4)
            m["bvb"] = (bv[None, DVS * r : DVS * (r + 1)] * WSCALE).astype(bf16)
        else:
            m["wv"] = wvs.astype(bf16)
            m["bvb"] = bv[None, DVS * r : DVS * (r + 1)].astype(bf16)
        in_maps.append(m)
    return in_maps


def assemble(results):
    """[8 x [512,1024]] core outputs -> [2,2048,1024]."""
    out = np.empty((B, S, D), np.float32)
    for c in range(NCORES):
        b, r = c // GROUP, c % GROUP
        for g in range(4):
            out[b, 512 * g + 128 * r : 512 * g + 128 * r + 128, :] = results[c][
                "out"
            ][128 * g : 128 * (g + 1)]
    return out


def kernel(x, Wk, bk, Wv, bv, Wf, bf, _trace=False, _trace_cores=None):
    global _compiled
    if _compiled is None:
        _compiled = build_program()
    nc = _compiled
    in_maps = make_in_maps(x, Wk, bk, Wv, bv, Wf, bf)
    res = bass_utils.run_bass_kernel_spmd(
        nc,
        in_maps,
        core_ids=list(range(NCORES)),
        trace=_trace,
        trace_cores=_trace_cores,
    )
    out = assemble(res.results)
    kernel.last_result = res
    return out
